# revision 9
# baseline (speedup 1.0000x reference)
"""CASSI forward A^T(A(x)) kernel for Trainium2, 8-core data parallel.

Reference computation (independent per batch b and row m):
    y1[l, n]  = x[b, l, m, n] * phi[l, m, n]
    y2[j]     = sum_l y1[l, j - 2l]              (j in [0, 310))
    out[l, n] = phi[l, m, n] * y2[2l + n]

On-chip layout: partitions = rows m (two 128-row tiles per batch), free
dim = (l, n).  The 28-band shift-scatter-add runs as a 5-level binary tree
of strided DVE adds over scratch tiles laid out with small zero gaps
between paired bands, so each tree level is a single wide strided
tensor_tensor op whose shifted operand reads zeros where a block has no
data.  Gaps are memset once at kernel start; level ops rewrite only data
regions.  Everything stays on the DVE: GpSimd shares the DVE's SBUF port
(concurrent Q7 traffic inflates every DVE op ~20%), so offloading tree
levels there is a net loss.

Uniform-slot layout: at every level, slot width = data width + next-level
shift, so in0's right-pad zeros and in1's left-pad zeros are the SAME gap
cells and every level op is a plain strided tensor_tensor:
  y1  band l (256) at 258*l                        gaps [256,258) per slot
  u   i=0..13 (258) at 262*i                       gaps [258,262)
  q   i=0..6  (262) at 278*i                       gaps [262,278), [1930,1938)
  o   i=0..2  (270) at 286*i                       gaps [270,286), [842,850)
  s   s0 (286) at 0, m1 (278) at 342               zeros [286,342)
  y2  (310) dense

Batch pairing: DVE time is free-dim elements plus a ~150ns fixed cost
per instruction, so consecutive batches share one tree pass over
double-width scratch tiles (second batch at +Y1_W/+U_W/... element
offsets, all even).  Each tree level is then a single op with a leading
batch dim (the 2x_1p perf mode keys on dtype/innermost step only, not
free-dim count), cutting the per-pair DVE op count from 16 to 10.  The
mask-muls and out-muls stay per-batch: merging them would need a
stride-0 phi broadcast dim.

Precision/layout strategy: all HBM traffic is fp16 (inputs are cast and
row-major-transposed to [.., M, L, N] on the host inside kernel(), the
output is cast/transposed back).  This halves HBM bytes and DVE cycles
(fp16 tensor_tensor runs in the 2x perf mode; all scratch offsets are
even-element, keeping operands 4-byte aligned) and makes every DMA a
dense fully-contiguous transfer.  fp16 accumulation of 28 bands keeps
worst-case relative error ~1e-3, far inside the 2e-2 gate.

Schedule: iteration 0's x lands in two half-tiles and phi0 is split
column-wise, all four transfers spread over both HWDGE queues, so the
first mask-mul starts ~4us earlier (Tile dependency tracking is
tile-granular, so a *shared* tile would wait for both halves anyway).
Batch 1's x is split across both queues to cover the ramp.  The last
out-mul/store runs as two band-halves on separate queues so the final
store drain overlaps the last DVE op.

Sharding: batch dim (32) split 4-per-core across 8 cores; phi replicated.
"""

import numpy as np

B, L, M, N = 32, 28, 256, 256
STRIDE = 2
NCORES = 8
BPC = B // NCORES            # batches per core
NOUT = N + STRIDE * (L - 1)  # 310
P = 128                      # partitions per row tile
LN = L * N                   # 7168
HALF = LN // 2               # 3584 (14 bands)
Y1_W = 258 * 28              # 7224, band l at 258*l, gaps [256,258) per slot
U_W = 262 * 14               # 3668, u_i at 262*i, gaps [258,262)
Q_W = 1938                   # q_i at 278*i (uniform); gaps [262,278), [1930,1938)
O_W = 850                    # o_i at 286*i; zeros [270,286)x2, [842,850)
S_W = 620                    # s0@0 (286), zeros [286,342), m1@342 (278)

_cached = {}


def _build_nc():
    import concourse.bass as bass
    import concourse.mybir as mybir
    from concourse.ap import AP
    from concourse.tile import TileContext

    f16 = mybir.dt.float16
    nc = bass.Bass()
    x = nc.dram_tensor("x", [BPC, M, LN], f16, kind="ExternalInput")
    phi = nc.dram_tensor("phi", [M, LN], f16, kind="ExternalInput")
    out = nc.dram_tensor("out", [BPC, M, LN], f16, kind="ExternalOutput")

    def sub(t, off, dims):
        """AP over tile t at element offset off with free dims [[step,count],..]."""
        full = t[:]
        return AP(full.tensor, full.offset + off,
                  [[full.ap[0][0], P]] + [list(d) for d in dims])

    def cols(hbm_ap, off, width):
        """Column slice [off, off+width) of a [128, LN] HBM access pattern."""
        return AP(hbm_ap.tensor, hbm_ap.offset + off,
                  [list(hbm_ap.ap[0]), [1, width]])

    with TileContext(nc) as tc:
        with (
            tc.tile_pool(name="phipool", bufs=1) as phipool,
            tc.tile_pool(name="xpool", bufs=1) as xpool,
            tc.tile_pool(name="scratch", bufs=1) as sp,
        ):
            # --- persistent tiles ------------------------------------------------
            phit = [phipool.tile([P, LN], f16, name=f"phi{pt}", tag=f"phi{pt}")
                    for pt in range(M // P)]
            xts = [xpool.tile([P, LN], f16, name=f"xt{i}", tag=f"xt{i}")
                   for i in range(3)]
            x0h = [xpool.tile([P, HALF], f16, name=f"x0{i}", tag=f"x0{i}")
                   for i in range(2)]  # iteration 0's x, as two half tiles
            ots_ = [xpool.tile([P, LN], f16, name=f"ou{i}", tag=f"ou{i}")
                    for i in range(2)]
            # double-width scratch: batch pair (ba, bb), bb at +W offsets
            y1t = sp.tile([P, 2 * Y1_W], f16, name="y1", tag="y1")
            ut = sp.tile([P, 2 * U_W], f16, name="u", tag="u")
            qt = sp.tile([P, 2 * Q_W], f16, name="q", tag="q")
            ot = sp.tile([P, 2 * O_W], f16, name="o", tag="o")
            st = sp.tile([P, 2 * S_W], f16, name="s", tag="s")
            y2t = sp.tile([P, 2 * NOUT], f16, name="y2", tag="y2")

            # --- one-time zero-gap memsets (never written afterwards) ------------
            for h in range(2):
                nc.vector.memset(sub(y1t, Y1_W * h + 256, [[258, 28], [1, 2]]), 0.0)
                nc.vector.memset(sub(ut, U_W * h + 258, [[262, 14], [1, 4]]), 0.0)
                nc.vector.memset(sub(qt, Q_W * h + 262, [[278, 6], [1, 16]]), 0.0)
                nc.vector.memset(sub(qt, Q_W * h + 1930, [[1, 8]]), 0.0)
                nc.vector.memset(sub(ot, O_W * h + 270, [[286, 2], [1, 16]]), 0.0)
                nc.vector.memset(sub(ot, O_W * h + 842, [[1, 8]]), 0.0)
                nc.vector.memset(sub(st, S_W * h + 286, [[1, 56]]), 0.0)

            # --- startup loads: batch 0 split across both queues -----------------
            nc.sync.dma_start(out=sub(phit[0], 0, [[1, HALF]]),
                              in_=cols(phi[0:P], 0, HALF))
            nc.scalar.dma_start(out=sub(phit[0], HALF, [[1, HALF]]),
                                in_=cols(phi[0:P], HALF, HALF))
            nc.sync.dma_start(out=x0h[0][:], in_=cols(x[0][0:P], 0, HALF))
            nc.scalar.dma_start(out=x0h[1][:], in_=cols(x[0][0:P], HALF, HALF))

            npair = M // P * BPC // 2  # 4 pairs of batches
            for pr in range(npair):
                pt, ba = (pr * 2) // BPC, (pr * 2) % BPC
                it_a, it_b = 2 * pr, 2 * pr + 1
                xta, xtb = xts[it_a % 3], xts[it_b % 3]
                # --- loads ------------------------------------------------------
                if pr == 0:
                    # batch 1's x split across both queues (DVE still ramping)
                    nc.sync.dma_start(out=sub(xtb, 0, [[1, HALF]]),
                                      in_=cols(x[1][0:P], 0, HALF))
                    nc.scalar.dma_start(out=sub(xtb, HALF, [[1, HALF]]),
                                        in_=cols(x[1][0:P], HALF, HALF))
                else:
                    for bi, xt in ((ba, xta), (ba + 1, xtb)):
                        nc.scalar.dma_start(
                            out=xt[:], in_=x[bi][pt * P: (pt + 1) * P],
                        )
                # --- mask-muls (per batch) --------------------------------------
                if pr == 0:
                    for h in range(2):
                        nc.vector.tensor_mul(
                            out=sub(y1t, 258 * 14 * h, [[258, 14], [1, 256]]),
                            in0=sub(x0h[h], 0, [[256, 14], [1, 256]]),
                            in1=sub(phit[pt], HALF * h, [[256, 14], [1, 256]]),
                        )
                else:
                    nc.vector.tensor_mul(
                        out=sub(y1t, 0, [[258, 28], [1, 256]]),
                        in0=sub(xta, 0, [[256, 28], [1, 256]]),
                        in1=sub(phit[pt], 0, [[256, 28], [1, 256]]),
                    )
                nc.vector.tensor_mul(
                    out=sub(y1t, Y1_W, [[258, 28], [1, 256]]),
                    in0=sub(xtb, 0, [[256, 28], [1, 256]]),
                    in1=sub(phit[pt], 0, [[256, 28], [1, 256]]),
                )
                # --- shared tree pass, batch dim leading ------------------------
                # L1: 2x14 pair-sums -> u
                nc.vector.tensor_add(
                    out=sub(ut, 0, [[U_W, 2], [262, 14], [1, 258]]),
                    in0=sub(y1t, 0, [[Y1_W, 2], [516, 14], [1, 258]]),
                    in1=sub(y1t, 256, [[Y1_W, 2], [516, 14], [1, 258]]),
                )
                # L2: 2x7 quad-sums -> q
                nc.vector.tensor_add(
                    out=sub(qt, 0, [[Q_W, 2], [278, 7], [1, 262]]),
                    in0=sub(ut, 0, [[U_W, 2], [524, 7], [1, 262]]),
                    in1=sub(ut, 258, [[U_W, 2], [524, 7], [1, 262]]),
                )
                # L3: 2x3 oct-sums -> o
                nc.vector.tensor_add(
                    out=sub(ot, 0, [[O_W, 2], [286, 3], [1, 270]]),
                    in0=sub(qt, 0, [[Q_W, 2], [556, 3], [1, 270]]),
                    in1=sub(qt, 270, [[Q_W, 2], [556, 3], [1, 270]]),
                )
                # L4: s0 = o0 + shift16(o1); m1 = o2 + shift16(q6)
                nc.vector.tensor_add(
                    out=sub(st, 0, [[S_W, 2], [1, 286]]),
                    in0=sub(ot, 0, [[O_W, 2], [1, 286]]),
                    in1=sub(ot, 270, [[O_W, 2], [1, 286]]),
                )
                nc.vector.tensor_add(
                    out=sub(st, 342, [[S_W, 2], [1, 278]]),
                    in0=sub(ot, 572, [[O_W, 2], [1, 278]]),
                    in1=sub(qt, 1652, [[Q_W, 2], [1, 278]]),
                )
                # L5: y2 = s0 + shift32(m1)
                nc.vector.tensor_add(
                    out=sub(y2t, 0, [[NOUT, 2], [1, 310]]),
                    in0=sub(st, 0, [[S_W, 2], [1, 310]]),
                    in1=sub(st, 310, [[S_W, 2], [1, 310]]),
                )
                # --- out-muls + stores (per batch) ------------------------------
                for h, bi in ((0, ba), (1, ba + 1)):
                    outt = ots_[h]
                    o_hbm = out[bi][pt * P: (pt + 1) * P]
                    if pr == npair - 1 and h == 1:
                        # very last batch: two half-band out-muls, each store
                        # firing as its half completes (on separate rings)
                        for hh, eng in ((0, nc.sync), (1, nc.scalar)):
                            nc.vector.tensor_mul(
                                out=sub(outt, HALF * hh, [[256, 14], [1, 256]]),
                                in0=sub(y2t, NOUT + 28 * hh, [[2, 14], [1, 256]]),
                                in1=sub(phit[pt], HALF * hh, [[256, 14], [1, 256]]),
                            )
                            eng.dma_start(
                                out=cols(o_hbm, HALF * hh, HALF),
                                in_=sub(outt, HALF * hh, [[1, HALF]]),
                            )
                    else:
                        nc.vector.tensor_mul(
                            out=sub(outt, 0, [[256, 28], [1, 256]]),
                            in0=sub(y2t, NOUT * h, [[2, 28], [1, 256]]),
                            in1=sub(phit[pt], 0, [[256, 28], [1, 256]]),
                        )
                        nc.sync.dma_start(out=o_hbm, in_=outt[:])
                        if pr == 0 and h == 0:
                            # phi tile 1 in the store queue's idle window
                            nc.sync.dma_start(out=phit[1][:], in_=phi[P: 2 * P])
    _split_excess_waits(nc, mybir)
    return nc


def _split_excess_waits(nc, mybir):
    """Move all-but-one semaphore waits off capacity-limited instructions.

    The TRN2 ISA packs sync commands into each 64B instruction; multi-dim
    TT/DMA encodings have room for only one wait, and walrus codegen dies
    with "Too many sync wait commands" instead of splitting.  A standalone
    EventSemaphore on the same engine right before the op is semantically
    identical (the sequencer executes both in order)."""
    ctr = 0
    for bb in nc.m.functions[0].blocks:
        new = []
        for ins in bb.instructions:
            si = ins.sync_info
            waits = list(si.on_wait) if si is not None and si.on_wait else []
            if len(waits) > 1:
                for w in waits[:-1]:
                    ctr += 1
                    new.append(mybir.InstEventSemaphore(
                        name=f"wsplit-{ctr}",
                        engine=ins.engine,
                        sync_info=mybir.SyncInfo(on_wait=[w], on_update=[]),
                    ))
                ins.sync_info = mybir.SyncInfo(
                    on_wait=[waits[-1]],
                    on_update=list(si.on_update or []),
                )
            new.append(ins)
        bb.instructions = new


def _get_nc():
    if "nc" not in _cached:
        _cached["nc"] = _build_nc()
    return _cached["nc"]


def _prep_in_maps(x: np.ndarray, phi: np.ndarray) -> list[dict]:
    """Shard batch across cores; cast to fp16 and transpose to [.., M, L, N]
    row-major so every device DMA is dense and contiguous."""
    phi_t = phi.transpose(1, 0, 2).reshape(M, LN).astype(np.float16, order="C")
    in_maps = []
    for c in range(NCORES):
        xs = (x[c * BPC: (c + 1) * BPC]
              .transpose(0, 2, 1, 3)
              .reshape(BPC, M, LN)
              .astype(np.float16, order="C"))
        in_maps.append({"x": xs, "phi": phi_t})
    return in_maps


def _postprocess(outs: list[np.ndarray]) -> np.ndarray:
    """Invert the device layout: fp16 [BPC, M, L*N] shards -> f32 [B,L,M,N]."""
    full = np.empty((B, L, M, N), dtype=np.float32)
    for c, o in enumerate(outs):
        o = np.asarray(o).reshape(BPC, M, L, N).astype(np.float32)
        full[c * BPC: (c + 1) * BPC] = o.transpose(0, 2, 1, 3)
    return full


def kernel(x: np.ndarray, phi: np.ndarray) -> np.ndarray:
    from concourse.bass_utils import run_bass_kernel_spmd

    x = np.asarray(x, dtype=np.float32)
    phi = np.asarray(phi, dtype=np.float32)
    assert x.shape == (B, L, M, N) and phi.shape == (L, M, N)

    nc = _get_nc()
    in_maps = _prep_in_maps(x, phi)
    res = run_bass_kernel_spmd(nc, in_maps, core_ids=list(range(NCORES)))
    return _postprocess([res.results[c]["out"] for c in range(NCORES)])
